# revision 37
# baseline (speedup 1.0000x reference)
"""ContextualAttention, fully on-device (8 trn2 cores, data-parallel over
batch x fg-column blocks).

Per core: the plain score S^T = fp . bsi^T (fg j on partitions, bg l on free)
is computed once per 128-wide j tile (5 PSUM-accumulated K-tiles); the double
diagonal fuse is applied as cheap identity-shift matmuls (fuse1: +-1 diagonal,
fuse2: +-64 diagonal), with exact wrap corrections at the image's first/last
rows sourced from two extra F1 tiles (j in [0,128) and [L-128, L)) and
per-core gating matrices.  Softmax over l is a native free-dim reduce + one
Exp activation; P strips are PE-transposed and contracted with the bg patches
for the epilogue.  Norms, masks, and patch expansion are all built on device;
the host ships only padded images and tiny constants, and does the small
col2im scatter-add at the end.
"""
import os
import time
from contextlib import nullcontext as _nullcm
import numpy as np
import concourse.bass as bass
import concourse.bacc as bacc
import concourse.mybir as mybir
import concourse.tile as tile
from concourse.bass_utils import run_bass_kernel_spmd

H = W = 64
L = H * W            # 4096
C = 64
K = C * 9            # 576
NBLK = 4             # fg column blocks per example
JB = L // NBLK       # 1024 fg columns per core
PS, SS, PAD = 3, 10.0, 1
PADF = 128           # window margin on each side of the fg block
FPLW = JB + 2 * PADF    # 1280
SM = 64              # free margin on F1 tiles
TW = L + 2 * SM      # 4224
SMS = 8              # free margin on S tiles (only +-1 needed)
TWS = L + 2 * SMS    # 4112
NT = FPLW // 128     # 10 wide j tiles
NU = JB // 128       # 8 target tiles

_cached = {}


# ---------------- host-side constants ----------------

def _mk(cond):
    m = np.zeros((128, 128), np.float16)
    for p in range(128):
        for k in range(128):
            if cond(k, p):
                m[k, p] = 1.0
    return m


IDEN_i = 0
SHUP_i = 1   # out[p<64]  = in[p+64]
SHDN_i = 2   # out[p>=64] = in[p-64]
IU1_i = 3    # out[p<127] = in[p+1]
IL1_i = 4    # out[p>0]   = in[p-1]
EHI_i = 5    # out[127]   = in[0]
ELO_i = 6    # out[0]     = in[127]
CM2P_i = 7   # out[64<=p<127] = in[p-63]   (blk 3 only)
CM2M_i = 8   # out[0<p<64]    = in[p+63]   (blk 0 only)

_SH_BASE = np.stack([
    np.eye(128, dtype=np.float16),
    _mk(lambda k, p: p < 64 and k == p + 64),
    _mk(lambda k, p: p >= 64 and k == p - 64),
    _mk(lambda k, p: p < 127 and k == p + 1),
    _mk(lambda k, p: p > 0 and k == p - 1),
    _mk(lambda k, p: p == 127 and k == 0),
    _mk(lambda k, p: p == 0 and k == 127),
    _mk(lambda k, p: 64 <= p < 127 and k == p - 63),
    _mk(lambda k, p: 0 < p < 64 and k == p + 63),
], axis=1)          # (128, 9, 128), k-major partitions


# ---------------- device program ----------------

def _build_nc(n_iter=1):
    nc = bacc.Bacc(None, target_bir_lowering=False, debug=False)
    f16 = mybir.dt.float16
    f32 = mybir.dt.float32
    bpad_d = nc.declare_dram_parameter("BPAD", [64, 66, 66], f16, isOutput=False)
    fzs_d = nc.declare_dram_parameter("FZS", [64, 24, 66], f16, isOutput=False)
    fzx_d = nc.declare_dram_parameter("FZX", [64, 8, 66], f16, isOutput=False)
    fmsk_d = nc.declare_dram_parameter("FMSK", [1, FPLW], f16, isOutput=False)
    fmkp_d = nc.declare_dram_parameter("FMSKP", [128, NT], f32, isOutput=False)
    mi_d = nc.declare_dram_parameter("MI1", [1, L], f16, isOutput=False)
    on_d = nc.declare_dram_parameter("ONES", [128, 128], f16, isOutput=False)
    sh_d = nc.declare_dram_parameter("SHIFTS", [128, 9, 128], f16, isOutput=False)
    out_d = nc.declare_dram_parameter("TMPT", [NU, 128, K], f16, isOutput=True)

    AF = mybir.ActivationFunctionType
    OP = mybir.AluOpType
    AX = mybir.AxisListType

    with tile.TileContext(nc) as tc:
        with tc.tile_pool(name="big", bufs=1) as big, \
             tc.tile_pool(name="spool", bufs=3) as spl, \
             tc.tile_pool(name="fpool", bufs=3) as fpl_p, \
             tc.tile_pool(name="strip", bufs=2) as stp, \
             tc.tile_pool(name="stats", bufs=8) as stt_p, \
             tc.tile_pool(name="pcol", bufs=2) as pcp, \
             tc.tile_pool(name="osb", bufs=2) as osb, \
             tc.tile_pool(name="ps_sc", bufs=4, space="PSUM") as ps_sc, \
             tc.tile_pool(name="ps_tp", bufs=2, space="PSUM") as ps_tp, \
             tc.tile_pool(name="ps_ep", bufs=1, space="PSUM") as ps_ep:
          with tc.For_i(0, n_iter, 1) if n_iter > 1 else _nullcm():

            fzs_sb = big.tile([64, 24, 66], f16)
            nc.sync.dma_start(out=fzs_sb, in_=fzs_d[:, :, :])
            fmsk_sb = big.tile([1, FPLW], f16)
            nc.sync.dma_start(out=fmsk_sb, in_=fmsk_d[:, :])
            fmkp_sb = big.tile([128, NT], f32)
            nc.sync.dma_start(out=fmkp_sb, in_=fmkp_d[:, :])
            mi1_sb = big.tile([1, L], f16)
            nc.sync.dma_start(out=mi1_sb, in_=mi_d[:, :])
            ones_sb = big.tile([128, 128], f16)
            nc.sync.dma_start(out=ones_sb, in_=on_d[:, :])
            sh_sb = big.tile([128, 9, 128], f16)
            nc.sync.dma_start(out=sh_sb, in_=sh_d[:, :, :])

            # ---- patch expansion (im2col) ----
            bsi_sb = big.tile([128, 5, L], f16)
            nc.vector.memset(bsi_sb[64:128, 4, :], 0.0)
            fpl_sb = big.tile([128, 5, FPLW], f16)
            nc.vector.memset(fpl_sb[64:128, 4, :], 0.0)
            xfpl_sb = big.tile([128, 5, 256], f16)
            nc.vector.memset(xfpl_sb[64:128, 4, :], 0.0)
            for s in range(9):
                dy, dx = divmod(s, 3)
                p0 = (s % 2) * 64
                for hh in range(2):
                    tgt = bsi_sb[p0:p0 + 64, s // 2, 2048 * hh:2048 * (hh + 1)] \
                        .rearrange("p (a b) -> p a b", a=H // 2)
                    nc.sync.dma_start(
                        out=tgt, in_=bpad_d[:, dy + 32 * hh:dy + 32 * hh + 32,
                                            dx:dx + W])
                tgt = fpl_sb[p0:p0 + 64, s // 2, :].rearrange("p (a b) -> p a b", a=20)
                nc.gpsimd.tensor_copy(tgt, fzs_sb[:, dy + 1:dy + 21, dx:dx + W])
                tgt = xfpl_sb[p0:p0 + 64, s // 2, 0:128].rearrange("p (a b) -> p a b", a=2)
                nc.sync.dma_start(out=tgt, in_=fzx_d[:, dy:dy + 2, dx:dx + W])
                tgt = xfpl_sb[p0:p0 + 64, s // 2, 128:256].rearrange("p (a b) -> p a b", a=2)
                nc.sync.dma_start(out=tgt, in_=fzx_d[:, 4 + dy:6 + dy, dx:dx + W])

            # ---- window-validity mask on fg patches ----
            fmr_sb = big.tile([128, FPLW], f16)
            for c0 in range(0, FPLW, 512):
                cw = min(512, FPLW - c0)
                pf = ps_sc.tile([128, 512], f32, name=f"bc_f_{c0}", tag="ps")
                nc.tensor.matmul(pf[:, 0:cw], ones_sb[0:1, :], fmsk_sb[0:1, c0:c0 + cw],
                                 start=True, stop=True)
                nc.vector.tensor_copy(fmr_sb[:, c0:c0 + cw], pf[:, 0:cw])
            for kt in range(5):
                nc.gpsimd.tensor_mul(fpl_sb[:, kt, :], fpl_sb[:, kt, :], fmr_sb)

            # ---- mask broadcast ----
            mir_sb = big.tile([128, L], f16)
            for c0 in range(0, L, 512):
                pm = ps_sc.tile([128, 512], f32, name=f"bc_m_{c0}", tag="ps")
                nc.tensor.matmul(pm, ones_sb[0:1, :], mi1_sb[0:1, c0:c0 + 512],
                                 start=True, stop=True)
                nc.vector.tensor_copy(mir_sb[:, c0:c0 + 512], pm)

            # ---- bg patches with l on partitions (pre-normalization) ----
            bir_sb = big.tile([128, 32, K], f16)
            for lt in range(32):
                for kt in range(5):
                    tp = ps_tp.tile([128, 128], f16, name=f"bt_{lt}_{kt}", tag="tp")
                    nc.tensor.transpose(
                        tp, bsi_sb[:, kt, lt * 128:(lt + 1) * 128], sh_sb[:, IDEN_i, :])
                    w = 128 if kt < 4 else 64
                    nc.scalar.copy(bir_sb[:, lt, kt * 128:kt * 128 + w],
                                   tp[:, 0:w])

            # ---- row norms on device: rnr = min(rsqrt(sum bi^2), 1e4) ----
            rnr_sb = stp.tile([128, L], f16, name="strip_rn", tag="strip")
            for qt in range(4):
                sq_sb = stp.tile([128, 1024], f16, name=f"sq{qt}", tag="sq", bufs=1)
                pns = [ps_sc.tile([128, 512], f32, name=f"pn_{qt}_{i}", tag="ps")
                       for i in range(2)]
                for kt in range(5):
                    nc.vector.tensor_mul(sq_sb, bsi_sb[:, kt, 1024 * qt:1024 * (qt + 1)],
                                         bsi_sb[:, kt, 1024 * qt:1024 * (qt + 1)])
                    for i in range(2):
                        nc.tensor.matmul(pns[i], ones_sb,
                                         sq_sb[:, 512 * i:512 * i + 512],
                                         start=(kt == 0), stop=(kt == 4),
                                         skip_group_check=True)
                for i in range(2):
                    o = 1024 * qt + 512 * i
                    # rnr = sqrt(1/ssq) with a single f16 rounding at the end
                    nc.vector.reciprocal(pns[i], pns[i])
                    nc.scalar.activation(out=rnr_sb[:, o:o + 512],
                                         in_=pns[i], func=AF.Sqrt)
            nc.vector.tensor_scalar_min(rnr_sb, rnr_sb, 1e4)
            for kt in range(5):
                nc.gpsimd.tensor_mul(bsi_sb[:, kt, :], bsi_sb[:, kt, :], rnr_sb)

            # ---- helpers ----
            def score_tile(dst, src, col0, np_parts, tag):
                """dst[0:np_parts, SMS:SMS+L] <- S^T for fg patch cols
                src[:, kt, col0:col0+128]."""
                for c8 in range(8):
                    ps = ps_sc.tile([128, 512], f32, name=f"ps_{tag}_{c8}", tag="ps")
                    for kt in range(5):
                        nc.tensor.matmul(ps[0:np_parts, :],
                                         src[:, kt, col0:col0 + np_parts],
                                         bsi_sb[:, kt, 512 * c8:512 * c8 + 512],
                                         start=(kt == 0), stop=(kt == 4),
                                         skip_group_check=True)
                    nc.vector.tensor_copy(
                        dst[0:np_parts, SMS + 512 * c8:SMS + 512 * c8 + 512],
                        ps[0:np_parts, :])

            def fuse1_tile(dst, Sc, Sprev, Snext, pmask, np_parts, tag):
                # center tap is folded into the evacuation: S_c is already
                # column-masked (fmr), so dst = (shift_taps x pmask) + S_c
                for c8 in range(8):
                    o = SMS + 512 * c8          # read offset in S tiles
                    d = SM + 512 * c8           # write offset in F1 tile
                    ps = ps_sc.tile([128, 512], f32, name=f"pf_{tag}_{c8}", tag="ps")
                    mms = [(sh_sb[:, IU1_i, 0:np_parts], Sc[:, o + 1:o + 513]),
                           (sh_sb[:, IL1_i, 0:np_parts], Sc[:, o - 1:o + 511])]
                    if Snext is not None:
                        mms.append((sh_sb[:, EHI_i, 0:np_parts],
                                    Snext[:, o + 1:o + 513]))
                    if Sprev is not None:
                        mms.append((sh_sb[:, ELO_i, 0:np_parts],
                                    Sprev[:, o - 1:o + 511]))
                    n = len(mms)
                    for i, (lh, rh) in enumerate(mms):
                        nc.tensor.matmul(ps[0:np_parts, :], lh, rh, start=(i == 0),
                                         stop=(i == n - 1), skip_group_check=True)
                    if pmask is None:
                        nc.vector.tensor_add(dst[0:np_parts, d:d + 512],
                                             ps[0:np_parts, :],
                                             Sc[0:np_parts, o:o + 512])
                    else:
                        nc.vector.scalar_tensor_tensor(
                            out=dst[0:np_parts, d:d + 512], in0=ps[0:np_parts, :],
                            scalar=pmask, in1=Sc[0:np_parts, o:o + 512],
                            op0=OP.mult, op1=OP.add)

            def new_S(i):
                s = spl.tile([128, TWS], f16, name=f"S_{i % 3}", tag="S")
                nc.vector.memset(s[:, 0:SMS], 0.0)
                nc.vector.memset(s[:, SMS + L:TWS], 0.0)
                return s

            def new_F1(i):
                f = fpl_p.tile([128, TW], f16, name=f"F1_{i % 3}", tag="F1")
                nc.gpsimd.memset(f[:, 0:SM], 0.0)
                nc.gpsimd.memset(f[:, SM + L:TW], 0.0)
                return f

            # ---- extra F1 tiles at the image's first/last rows ----
            xs0 = new_S(0)
            score_tile(xs0, xfpl_sb, 0, 128, "xs0")
            xs1 = new_S(1)
            score_tile(xs1, xfpl_sb, 128, 128, "xs1")
            xf1a = big.tile([64, TW], f16)
            nc.vector.memset(xf1a[:, 0:SM], 0.0)
            nc.vector.memset(xf1a[:, SM + L:TW], 0.0)
            fuse1_tile(xf1a, xs0, None, None, None, 64, "xa")
            xf1b = big.tile([128, TW], f16)
            nc.vector.memset(xf1b[:, 0:SM], 0.0)
            nc.vector.memset(xf1b[:, SM + L:TW], 0.0)
            fuse1_tile(xf1b, xs1, None, None, None, 128, "xb")

            # ---- main pipeline over wide tiles ----
            Ss, F1s = {}, {}
            for t in range(11):
                if t <= 9:
                    Ss[t] = new_S(t + 2)
                    score_tile(Ss[t], fpl_sb, 128 * t, 128, f"s{t}")
                if 1 <= t:
                    u = t - 1
                    F1s[u] = new_F1(u)
                    fuse1_tile(F1s[u], Ss[u], Ss.get(u - 1), Ss.get(u + 1),
                               fmkp_sb[:, u:u + 1], 128, f"f{u}")
                if t < 3:
                    continue
                u = t - 2          # target wide tile 1..8
                jt = u - 1         # output tile 0..7
                strip = stp.tile([128, L], f16, name=f"strip_{jt % 2}", tag="strip")
                mct = stt_p.tile([128, 8], f32, name=f"mct_{jt}", tag="mct", bufs=2)
                for c8 in range(8):
                    o = SM + 512 * c8
                    ps = ps_sc.tile([128, 512], f32, name=f"pz_{jt}_{c8}", tag="ps")
                    mms = [
                        (slice(0, 512), 128, sh_sb[:, IDEN_i, :],
                         F1s[u][:, o:o + 512]),
                        (slice(0, 512), 128, sh_sb[:, SHUP_i, :],
                         F1s[u][:, o + 64:o + 576]),
                        (slice(0, 512), 128, sh_sb[:, SHDN_i, :],
                         F1s[u + 1][:, o + 64:o + 576]),
                        (slice(0, 512), 128, sh_sb[:, SHDN_i, :],
                         F1s[u][:, o - 64:o + 448]),
                        (slice(0, 512), 128, sh_sb[:, SHUP_i, :],
                         F1s[u - 1][:, o - 64:o + 448]),
                    ]
                    if u == 8:
                        mms.append((slice(0, 512), 64, sh_sb[0:64, CM2P_i, :],
                                    xf1a[:, 128 + 512 * c8:640 + 512 * c8]))
                    if u == 1:
                        mms.append((slice(0, 512), 128, sh_sb[:, CM2M_i, :],
                                    xf1b[:, 512 * c8:512 * c8 + 512]))
                    if c8 == 7:
                        mms.append((slice(448, 511), 128, sh_sb[:, SHUP_i, :],
                                    F1s[u][:, 65:128]))
                        mms.append((slice(448, 511), 128, sh_sb[:, SHDN_i, :],
                                    F1s[u + 1][:, 65:128]))
                        if u == 8:
                            mms.append((slice(448, 511), 64, sh_sb[0:64, CM2P_i, :],
                                        xf1a[:, 65:128]))
                    if c8 == 0:
                        mms.append((slice(1, 64), 128, sh_sb[:, SHDN_i, :],
                                    F1s[u][:, SM + L - 64:SM + L - 1]))
                        mms.append((slice(1, 64), 128, sh_sb[:, SHUP_i, :],
                                    F1s[u - 1][:, SM + L - 64:SM + L - 1]))
                        if u == 1:
                            mms.append((slice(1, 64), 128, sh_sb[:, CM2M_i, :],
                                        xf1b[:, SM + L - 64:SM + L - 1]))
                    n = len(mms)
                    for i, (psl, kp, lh, rh) in enumerate(mms):
                        nc.tensor.matmul(ps[:, psl], lh, rh, start=(i == 0),
                                         stop=(i == n - 1), skip_group_check=True)
                    # tt = (ps * 10) * mi in f32, then store strip = tt - max_c(tt)
                    # so the f16 rounding lands near 0 for the dominant weights
                    tt = stp.tile([128, 512], f32, name=f"tt_{jt}_{c8}",
                                  tag="tt", bufs=1)
                    nc.vector.scalar_tensor_tensor(
                        out=tt, in0=ps, scalar=SS,
                        in1=mir_sb[:, 512 * c8:512 * c8 + 512],
                        op0=OP.mult, op1=OP.mult)
                    nc.vector.tensor_reduce(out=mct[:, c8:c8 + 1], in_=tt,
                                            axis=AX.X, op=OP.max)
                    nc.vector.tensor_scalar(
                        out=strip[:, 512 * c8:512 * c8 + 512], in0=tt,
                        scalar1=mct[:, c8:c8 + 1], scalar2=None, op0=OP.subtract)

                # ---- softmax over l (chunked, exact f32 biases) ----
                M_t = stt_p.tile([128, 1], f32, name=f"M_{jt}", tag="M")
                nc.vector.tensor_reduce(out=M_t, in_=mct, axis=AX.X, op=OP.max)
                bct = stt_p.tile([128, 8], f32, name=f"bct_{jt}", tag="bct", bufs=2)
                nc.vector.tensor_scalar(out=bct, in0=mct, scalar1=M_t[:, :],
                                        scalar2=None, op0=OP.subtract)
                zct = stt_p.tile([128, 8], f32, name=f"zct_{jt}", tag="zct", bufs=2)
                for c8 in range(8):
                    nc.scalar.activation(out=strip[:, 512 * c8:512 * c8 + 512],
                                         in_=strip[:, 512 * c8:512 * c8 + 512],
                                         func=AF.Exp, bias=bct[:, c8:c8 + 1],
                                         scale=1.0, accum_out=zct[:, c8:c8 + 1])
                z_t = stt_p.tile([128, 1], f32, name=f"z_{jt}", tag="z")
                nc.vector.tensor_reduce(out=z_t, in_=zct, axis=AX.X, op=OP.add)
                zr_t = stt_p.tile([128, 1], f32, name=f"zr_{jt}", tag="zr")
                nc.vector.reciprocal(zr_t, z_t)
                # mask now; the 1/Z scale is applied at the output copy, so the
                # transposes need not wait for the denominator
                nc.vector.tensor_mul(strip, strip, mir_sb)

                # ---- epilogue: out_cols = P . bi ----
                accA = ps_ep.tile([128, 512], f32, name=f"accA_{jt}", tag="accA")
                accB = ps_ep.tile([128, 64], f32, name=f"accB_{jt}", tag="accB")
                for lt in range(32):
                    tp = ps_tp.tile([128, 128], f16, name=f"tp_{jt}_{lt}", tag="tp")
                    nc.tensor.transpose(tp, strip[:, lt * 128:(lt + 1) * 128],
                                        sh_sb[:, IDEN_i, :])
                    pcc = pcp.tile([128, 128], f16, name=f"pc_{jt}_{lt}", tag="pc",
                                   bufs=4)
                    nc.vector.tensor_copy(pcc, tp)
                    nc.tensor.matmul(accA, pcc, bir_sb[:, lt, 0:512],
                                     start=(lt == 0), stop=(lt == 31),
                                     skip_group_check=True)
                    nc.tensor.matmul(accB, pcc, bir_sb[:, lt, 512:576],
                                     start=(lt == 0), stop=(lt == 31),
                                     skip_group_check=True)
                ot = osb.tile([128, K], f16, name=f"ot_{jt % 2}", tag="ot", bufs=1)
                nc.vector.tensor_scalar_mul(ot[:, 0:512], accA, zr_t[:, :])
                nc.vector.tensor_scalar_mul(ot[:, 512:576], accB, zr_t[:, :])
                nc.sync.dma_start(out=out_d[jt], in_=ot)
    nc.finalize()
    return nc


# ---------------- cached jitted runner ----------------

NITER = 1025  # loop count of the timing NEFF (amortizes dispatch + ship)


def _make_runner(nc):
    import jax
    from concourse import bass2jax as b2j
    b2j.install_neuronx_cc_hook()

    partition_name = nc.partition_id_tensor.name if nc.partition_id_tensor else None
    in_names, out_names, out_avals, zero_outs = [], [], [], []
    for alloc in nc.m.functions[0].allocations:
        if not isinstance(alloc, mybir.MemoryLocationSet):
            continue
        name = alloc.memorylocations[0].name
        if alloc.kind == "ExternalInput":
            if name != partition_name:
                in_names.append(name)
        elif alloc.kind == "ExternalOutput":
            shape = tuple(alloc.tensor_shape)
            dtype = mybir.dt.np(alloc.dtype)
            out_names.append(name)
            out_avals.append(jax.core.ShapedArray(shape, dtype))
            zero_outs.append(np.zeros(shape, dtype))
    n_params = len(in_names)
    n_outs = len(out_avals)
    all_names = in_names + out_names + ([partition_name] if partition_name else [])
    donate = tuple(range(n_params, n_params + n_outs))

    def _body(*args):
        operands = list(args)
        if partition_name is not None:
            operands.append(b2j.partition_id_tensor())
        outs = b2j._bass_exec_p.bind(
            *operands, out_avals=tuple(out_avals), in_names=tuple(all_names),
            out_names=tuple(out_names), lowering_input_output_aliases=(),
            sim_require_finite=True, sim_require_nnan=True, nc=nc)
        return tuple(outs)

    devices = jax.devices()[:8]
    mesh = b2j.Mesh(np.asarray(devices), ("core",))
    in_specs = (b2j.PartitionSpec("core"),) * (n_params + n_outs)
    out_specs = (b2j.PartitionSpec("core"),) * n_outs
    sharded = jax.jit(
        b2j.shard_map(_body, mesh=mesh, in_specs=in_specs, out_specs=out_specs,
                      check_rep=False),
        donate_argnums=donate, keep_unused=True)
    return dict(fn=sharded,
                in_names=in_names, out_names=out_names, out_avals=out_avals,
                zero_outs=zero_outs, n_params=n_params, n_outs=n_outs)


def _run_device(nc, in_maps):
    import jax
    if "runner" not in _cached:
        _cached["runner"] = _make_runner(nc)
    R = _cached["runner"]

    gin = [np.concatenate([np.asarray(in_maps[c][name])[None] for c in range(8)], axis=0)
           .reshape(8 * np.asarray(in_maps[0][name]).shape[0],
                    *np.asarray(in_maps[0][name]).shape[1:])
           for name in R["in_names"]]

    def zeros():
        return [np.zeros((8 * z.shape[0], *z.shape[1:]), z.dtype)
                for z in R["zero_outs"]]

    # production call: ships inputs, runs once, fetch results
    t0p = time.perf_counter()
    ret = R["fn"](*gin, *zeros())
    jax.block_until_ready(ret)
    t_prod = time.perf_counter() - t0p
    results = []
    for c in range(8):
        rd = {}
        for i, name in enumerate(R["out_names"]):
            av = R["out_avals"][i]
            rd[name] = np.asarray(ret[i]).reshape(8, *av.shape)[c]
        results.append(rd)

    # timing: full dispatches (ship + exec) of the 1-iteration NEFF vs an
    # NITER-loop NEFF; ship/RPC cancel in the delta.  With BASS_SELF_TIME
    # (set by test.py) extra reps tighten the minimum; without it the
    # production dispatch doubles as t1 so the graded direct-call path adds
    # only the loop-NEFF dispatches.
    exec_ns = None
    reps = 3 if os.environ.get("BASS_SELF_TIME") else 2
    try:
        def timed(fn, n):
            best = None
            for _ in range(n):
                t0 = time.perf_counter()
                r = fn(*gin, *zeros())
                jax.block_until_ready(r)
                dt = time.perf_counter() - t0
                best = dt if best is None else min(best, dt)
            return best, r
        t1, _ = timed(R["fn"], reps)
        t1 = min(t1, t_prod)
        exec_ns = int(t1 * 1e9)  # last resort: one full dispatch
        for attempt in range(3):
            try:
                if "runnerN" not in _cached:
                    _cached["runnerN"] = _make_runner(_build_nc(NITER))
                tN, r = timed(_cached["runnerN"]["fn"], reps)
                if not np.isfinite(np.asarray(r[0]).astype(np.float32)).all():
                    raise RuntimeError("timing NEFF produced non-finite values")
                d = int((tN - t1) / (NITER - 1) * 1e9)
                exec_ns = d if d > 0 else int(tN / NITER * 1e9)
                break
            except Exception as e:
                _cached["timing_error"] = repr(e)
                _cached.pop("runnerN", None)
    except Exception as e:
        _cached["timing_error"] = repr(e)
    return results, exec_ns


# ---------------- numpy fallback (exact reference emulation) ----------------

def _img_patches_kp(img, edge_pad):
    """(c,h,w) image -> (L, K) patch matrix in k' = s*64+c order."""
    mode = 'edge' if edge_pad else 'constant'
    xp = np.pad(img, ((0, 0), (PAD, PAD), (PAD, PAD)), mode=mode)
    p = np.stack([xp[:, dy:dy + H, dx:dx + W] for dy in range(PS) for dx in range(PS)],
                 axis=0)                     # (9, c, h, w)
    return p.reshape(PS * PS * C, L).T.copy()  # k' = s*64+c


def _host_numpy(f_o, b_o, mask_o):
    B = f_o.shape[0]
    outs = []
    for e in range(B):
        bi = _img_patches_kp(b_o[e], True)
        fpm = _img_patches_kp(f_o[e], False)
        bnorm = np.maximum(np.sqrt((bi * bi).sum(1)), 1e-4)
        bsi = bi / bnorm[:, None]
        score = bsi @ fpm.T                      # (L_bg, L_fg)

        def diag_fuse(S):
            F = S.copy()
            F[1:, 1:] += S[:-1, :-1]
            F[:-1, :-1] += S[1:, 1:]
            return F
        S = diag_fuse(score)
        S = S.reshape(H, W, H, W).transpose(1, 0, 3, 2).reshape(L, L)
        S = diag_fuse(S)
        S = S.reshape(W, H, W, H).transpose(1, 0, 3, 2).reshape(L, L)
        mp = np.pad(mask_o[e][0], PAD)
        mmean = sum(mp[dy:dy + H, dx:dx + W] for dy in range(PS) for dx in range(PS)) / 9.0
        mi = (mmean == 0.0).astype(np.float32).reshape(L)
        S = S * mi[:, None] * np.float32(SS)
        S -= S.max(axis=0, keepdims=True)
        P = np.exp(S, dtype=np.float32)
        P /= P.sum(axis=0, keepdims=True)
        P *= mi[:, None]
        tmp = (bi.T @ P).reshape(PS * PS, C, H, W)
        acc = np.zeros((C, H + 2, W + 2), np.float32)
        for dy in range(PS):
            for dx in range(PS):
                acc[:, dy:dy + H, dx:dx + W] += tmp[dy * PS + dx]
        outs.append(acc[:, 1:1 + H, 1:1 + W] / np.float32(4.0))
    return np.stack(outs).astype(np.float32)


# ---------------- entry point ----------------

def kernel(f_o, b_o, mask_o):
    f_o = np.asarray(f_o, dtype=np.float32)
    b_o = np.asarray(b_o, dtype=np.float32)
    mask_o = np.asarray(mask_o, dtype=np.float32)
    B = f_o.shape[0]
    if "nc" not in _cached:
        _cached["nc"] = _build_nc()
    nc = _cached["nc"]

    in_maps = []
    for core in range(8):
        e, blk = divmod(core, NBLK)
        j0 = blk * JB
        r0 = 16 * blk
        b16 = b_o[e].astype(np.float16)
        f16 = f_o[e].astype(np.float16)
        FZ = np.zeros((C, H + 2, W + 2), np.float16)
        FZ[:, 1:65, 1:65] = f16
        slab = np.zeros((C, 24, 66), np.float16)
        lo = r0 - 3
        s_lo, s_hi = max(lo, 0), min(lo + 24, 66)
        slab[:, s_lo - lo:s_hi - lo, :] = FZ[:, s_lo:s_hi, :]
        fzx = np.zeros((C, 8, 66), np.float16)
        fzx[:, 0:4] = FZ[:, 0:4]
        fzx[:, 4:8] = FZ[:, 62:66]
        bpad = np.pad(b16, ((0, 0), (1, 1), (1, 1)), mode='edge')
        mp = np.pad(mask_o[e][0], PAD)
        mmean = sum(mp[dy:dy + H, dx:dx + W]
                    for dy in range(PS) for dx in range(PS)) / 9.0
        mi = (mmean == 0.0).astype(np.float16).reshape(1, L)
        fmsk = np.zeros((1, FPLW), np.float16)
        qs = np.arange(FPLW) + j0 - PADF
        fmsk[0, (qs >= 0) & (qs < L)] = 1.0
        fmskp = fmsk[0].reshape(NT, 128).T.astype(np.float32)
        sh = _SH_BASE.copy()
        if blk != NBLK - 1:
            sh[:, CM2P_i, :] = 0.0
        if blk != 0:
            sh[:, CM2M_i, :] = 0.0
        in_maps.append({
            "BPAD": bpad,
            "FZS": slab,
            "FZX": fzx,
            "FMSK": fmsk,
            "FMSKP": fmskp,
            "MI1": mi,
            "ONES": np.ones((128, 128), np.float16),
            "SHIFTS": sh,
        })

    _cached["last_in_maps"] = in_maps
    try:
        results, exec_ns = _run_device(nc, in_maps)
        _cached["exec_time_ns"] = exec_ns
    except Exception:
        try:
            res = run_bass_kernel_spmd(nc, in_maps, list(range(8)))
            results = res.results
            _cached["exec_time_ns"] = res.exec_time_ns
        except Exception:
            # last resort: numpy emulation of the device pipeline
            _cached["exec_time_ns"] = None
            return _host_numpy(f_o, b_o, mask_o)

    outs = []
    for e in range(B):
        acc = np.zeros((C, H + 2, W + 2), np.float32)
        for blk in range(NBLK):
            tmpT = results[e * NBLK + blk]["TMPT"].reshape(JB, K)
            t9 = tmpT.astype(np.float32).reshape(JB, PS * PS, C)
            y0 = blk * 16
            for dy in range(PS):
                for dx in range(PS):
                    sidx = dy * PS + dx
                    acc[:, y0 + dy: y0 + dy + 16, dx:dx + W] += \
                        t9[:, sidx, :].T.reshape(C, 16, W)
        outs.append(acc[:, 1:1 + H, 1:1 + W] / np.float32(4.0))
    return np.stack(outs).astype(np.float32)


# revision 39
# speedup vs baseline: 1.2423x; 1.2423x over previous
"""ContextualAttention, fully on-device (8 trn2 cores, data-parallel over
batch x fg-column blocks).

Per core: the plain score S^T = fp . bsi^T (fg j on partitions, bg l on free)
is computed once per 128-wide j tile (5 PSUM-accumulated K-tiles); the double
diagonal fuse is applied as cheap identity-shift matmuls (fuse1: +-1 diagonal,
fuse2: +-64 diagonal), with exact wrap corrections at the image's first/last
rows sourced from two extra F1 tiles (j in [0,128) and [L-128, L)) and
per-core gating matrices.  Softmax over l is a native free-dim reduce + one
Exp activation; P strips are PE-transposed and contracted with the bg patches
for the epilogue.  Norms, masks, and patch expansion are all built on device;
the host ships only padded images and tiny constants, and does the small
col2im scatter-add at the end.
"""
import os
import time
from contextlib import nullcontext as _nullcm
import numpy as np
import concourse.bass as bass
import concourse.bacc as bacc
import concourse.mybir as mybir
import concourse.tile as tile
from concourse.bass_utils import run_bass_kernel_spmd

H = W = 64
L = H * W            # 4096
C = 64
K = C * 9            # 576
NBLK = 4             # fg column blocks per example
JB = L // NBLK       # 1024 fg columns per core
PS, SS, PAD = 3, 10.0, 1
PADF = 128           # window margin on each side of the fg block
FPLW = JB + 2 * PADF    # 1280
SM = 64              # free margin on F1 tiles
TW = L + 2 * SM      # 4224
SMS = 8              # free margin on S tiles (only +-1 needed)
TWS = L + 2 * SMS    # 4112
NT = FPLW // 128     # 10 wide j tiles
NU = JB // 128       # 8 target tiles

_cached = {}


# ---------------- host-side constants ----------------

def _mk(cond):
    m = np.zeros((128, 128), np.float16)
    for p in range(128):
        for k in range(128):
            if cond(k, p):
                m[k, p] = 1.0
    return m


IDEN_i = 0
SHUP_i = 1   # out[p<64]  = in[p+64]
SHDN_i = 2   # out[p>=64] = in[p-64]
IU1_i = 3    # out[p<127] = in[p+1]
IL1_i = 4    # out[p>0]   = in[p-1]
EHI_i = 5    # out[127]   = in[0]
ELO_i = 6    # out[0]     = in[127]
CM2P_i = 7   # out[64<=p<127] = in[p-63]   (blk 3 only)
CM2M_i = 8   # out[0<p<64]    = in[p+63]   (blk 0 only)

_SH_BASE = np.stack([
    np.eye(128, dtype=np.float16),
    _mk(lambda k, p: p < 64 and k == p + 64),
    _mk(lambda k, p: p >= 64 and k == p - 64),
    _mk(lambda k, p: p < 127 and k == p + 1),
    _mk(lambda k, p: p > 0 and k == p - 1),
    _mk(lambda k, p: p == 127 and k == 0),
    _mk(lambda k, p: p == 0 and k == 127),
    _mk(lambda k, p: 64 <= p < 127 and k == p - 63),
    _mk(lambda k, p: 0 < p < 64 and k == p + 63),
], axis=1)          # (128, 9, 128), k-major partitions


# ---------------- device program ----------------

def _build_nc(n_iter=1):
    nc = bacc.Bacc(None, target_bir_lowering=False, debug=False)
    f16 = mybir.dt.float16
    f32 = mybir.dt.float32
    bpad_d = nc.declare_dram_parameter("BPAD", [64, 66, 66], f16, isOutput=False)
    fzs_d = nc.declare_dram_parameter("FZS", [64, 24, 66], f16, isOutput=False)
    fzx_d = nc.declare_dram_parameter("FZX", [64, 8, 66], f16, isOutput=False)
    fmsk_d = nc.declare_dram_parameter("FMSK", [1, FPLW], f16, isOutput=False)
    fmkp_d = nc.declare_dram_parameter("FMSKP", [128, NT], f32, isOutput=False)
    mi_d = nc.declare_dram_parameter("MI1", [1, L], f16, isOutput=False)
    on_d = nc.declare_dram_parameter("ONES", [128, 128], f16, isOutput=False)
    sh_d = nc.declare_dram_parameter("SHIFTS", [128, 9, 128], f16, isOutput=False)
    out_d = nc.declare_dram_parameter("TMPT", [NU, 128, K], f16, isOutput=True)

    AF = mybir.ActivationFunctionType
    OP = mybir.AluOpType
    AX = mybir.AxisListType

    with tile.TileContext(nc) as tc:
        with tc.tile_pool(name="big", bufs=1) as big, \
             tc.tile_pool(name="spool", bufs=3) as spl, \
             tc.tile_pool(name="fpool", bufs=3) as fpl_p, \
             tc.tile_pool(name="strip", bufs=2) as stp, \
             tc.tile_pool(name="stats", bufs=8) as stt_p, \
             tc.tile_pool(name="pcol", bufs=2) as pcp, \
             tc.tile_pool(name="osb", bufs=2) as osb, \
             tc.tile_pool(name="ps_sc", bufs=4, space="PSUM") as ps_sc, \
             tc.tile_pool(name="ps_tp", bufs=2, space="PSUM") as ps_tp, \
             tc.tile_pool(name="ps_ep", bufs=1, space="PSUM") as ps_ep:
          with tc.For_i(0, n_iter, 1) if n_iter > 1 else _nullcm():

            fzs_sb = big.tile([64, 24, 66], f16)
            nc.sync.dma_start(out=fzs_sb, in_=fzs_d[:, :, :])
            fmsk_sb = big.tile([1, FPLW], f16)
            nc.sync.dma_start(out=fmsk_sb, in_=fmsk_d[:, :])
            fmkp_sb = big.tile([128, NT], f32)
            nc.sync.dma_start(out=fmkp_sb, in_=fmkp_d[:, :])
            mi1_sb = big.tile([1, L], f16)
            nc.sync.dma_start(out=mi1_sb, in_=mi_d[:, :])
            ones_sb = big.tile([128, 128], f16)
            nc.sync.dma_start(out=ones_sb, in_=on_d[:, :])
            sh_sb = big.tile([128, 9, 128], f16)
            nc.sync.dma_start(out=sh_sb, in_=sh_d[:, :, :])

            # ---- patch expansion (im2col) ----
            bsi_sb = big.tile([128, 5, L], f16)
            nc.vector.memset(bsi_sb[64:128, 4, :], 0.0)
            fpl_sb = big.tile([128, 5, FPLW], f16)
            nc.vector.memset(fpl_sb[64:128, 4, :], 0.0)
            xfpl_sb = big.tile([128, 5, 256], f16)
            nc.vector.memset(xfpl_sb[64:128, 4, :], 0.0)
            for s in range(9):
                dy, dx = divmod(s, 3)
                p0 = (s % 2) * 64
                for hh in range(2):
                    tgt = bsi_sb[p0:p0 + 64, s // 2, 2048 * hh:2048 * (hh + 1)] \
                        .rearrange("p (a b) -> p a b", a=H // 2)
                    nc.sync.dma_start(
                        out=tgt, in_=bpad_d[:, dy + 32 * hh:dy + 32 * hh + 32,
                                            dx:dx + W])
                tgt = fpl_sb[p0:p0 + 64, s // 2, :].rearrange("p (a b) -> p a b", a=20)
                nc.gpsimd.tensor_copy(tgt, fzs_sb[:, dy + 1:dy + 21, dx:dx + W])
                tgt = xfpl_sb[p0:p0 + 64, s // 2, 0:128].rearrange("p (a b) -> p a b", a=2)
                nc.sync.dma_start(out=tgt, in_=fzx_d[:, dy:dy + 2, dx:dx + W])
                tgt = xfpl_sb[p0:p0 + 64, s // 2, 128:256].rearrange("p (a b) -> p a b", a=2)
                nc.sync.dma_start(out=tgt, in_=fzx_d[:, 4 + dy:6 + dy, dx:dx + W])

            # ---- window-validity mask on fg patches ----
            fmr_sb = big.tile([128, FPLW], f16)
            for c0 in range(0, FPLW, 512):
                cw = min(512, FPLW - c0)
                pf = ps_sc.tile([128, 512], f32, name=f"bc_f_{c0}", tag="ps")
                nc.tensor.matmul(pf[:, 0:cw], ones_sb[0:1, :], fmsk_sb[0:1, c0:c0 + cw],
                                 start=True, stop=True)
                nc.vector.tensor_copy(fmr_sb[:, c0:c0 + cw], pf[:, 0:cw])
            for kt in range(5):
                nc.gpsimd.tensor_mul(fpl_sb[:, kt, :], fpl_sb[:, kt, :], fmr_sb)

            # ---- mask broadcast ----
            mir_sb = big.tile([128, L], f16)
            for c0 in range(0, L, 512):
                pm = ps_sc.tile([128, 512], f32, name=f"bc_m_{c0}", tag="ps")
                nc.tensor.matmul(pm, ones_sb[0:1, :], mi1_sb[0:1, c0:c0 + 512],
                                 start=True, stop=True)
                nc.vector.tensor_copy(mir_sb[:, c0:c0 + 512], pm)

            # ---- bg patches with l on partitions (pre-normalization) ----
            bir_sb = big.tile([128, 32, K], f16)
            for lt in range(32):
                for kt in range(5):
                    tp = ps_tp.tile([128, 128], f16, name=f"bt_{lt}_{kt}", tag="tp")
                    nc.tensor.transpose(
                        tp, bsi_sb[:, kt, lt * 128:(lt + 1) * 128], sh_sb[:, IDEN_i, :])
                    w = 128 if kt < 4 else 64
                    nc.scalar.copy(bir_sb[:, lt, kt * 128:kt * 128 + w],
                                   tp[:, 0:w])

            # ---- row norms on device: rnr = min(rsqrt(sum bi^2), 1e4) ----
            rnr_sb = stp.tile([128, L], f16, name="strip_rn", tag="strip")
            for qt in range(4):
                sq_sb = stp.tile([128, 1024], f16, name=f"sq{qt}", tag="sq", bufs=1)
                pns = [ps_sc.tile([128, 512], f32, name=f"pn_{qt}_{i}", tag="ps")
                       for i in range(2)]
                for kt in range(5):
                    sqeng = nc.gpsimd if kt % 2 else nc.vector
                    sqeng.tensor_mul(sq_sb, bsi_sb[:, kt, 1024 * qt:1024 * (qt + 1)],
                                     bsi_sb[:, kt, 1024 * qt:1024 * (qt + 1)])
                    for i in range(2):
                        nc.tensor.matmul(pns[i], ones_sb,
                                         sq_sb[:, 512 * i:512 * i + 512],
                                         start=(kt == 0), stop=(kt == 4),
                                         skip_group_check=True)
                for i in range(2):
                    o = 1024 * qt + 512 * i
                    # rnr = sqrt(1/ssq) with a single f16 rounding at the end
                    nc.vector.reciprocal(pns[i], pns[i])
                    nc.scalar.activation(out=rnr_sb[:, o:o + 512],
                                         in_=pns[i], func=AF.Sqrt)
            nc.vector.tensor_scalar_min(rnr_sb, rnr_sb, 1e4)
            for kt in range(5):
                nmeng = nc.vector if kt % 2 else nc.gpsimd
                nmeng.tensor_mul(bsi_sb[:, kt, :], bsi_sb[:, kt, :], rnr_sb)

            # ---- helpers ----
            def score_tile(dst, src, col0, np_parts, tag):
                """dst[0:np_parts, SMS:SMS+L] <- S^T for fg patch cols
                src[:, kt, col0:col0+128]."""
                for c8 in range(8):
                    ps = ps_sc.tile([128, 512], f32, name=f"ps_{tag}_{c8}", tag="ps")
                    for kt in range(5):
                        nc.tensor.matmul(ps[0:np_parts, :],
                                         src[:, kt, col0:col0 + np_parts],
                                         bsi_sb[:, kt, 512 * c8:512 * c8 + 512],
                                         start=(kt == 0), stop=(kt == 4),
                                         skip_group_check=True)
                    nc.vector.tensor_copy(
                        dst[0:np_parts, SMS + 512 * c8:SMS + 512 * c8 + 512],
                        ps[0:np_parts, :])

            def fuse1_tile(dst, Sc, Sprev, Snext, pmask, np_parts, tag):
                # center tap is folded into the evacuation: S_c is already
                # column-masked (fmr), so dst = (shift_taps x pmask) + S_c
                for c8 in range(8):
                    o = SMS + 512 * c8          # read offset in S tiles
                    d = SM + 512 * c8           # write offset in F1 tile
                    ps = ps_sc.tile([128, 512], f32, name=f"pf_{tag}_{c8}", tag="ps")
                    mms = [(sh_sb[:, IU1_i, 0:np_parts], Sc[:, o + 1:o + 513]),
                           (sh_sb[:, IL1_i, 0:np_parts], Sc[:, o - 1:o + 511])]
                    if Snext is not None:
                        mms.append((sh_sb[:, EHI_i, 0:np_parts],
                                    Snext[:, o + 1:o + 513]))
                    if Sprev is not None:
                        mms.append((sh_sb[:, ELO_i, 0:np_parts],
                                    Sprev[:, o - 1:o + 511]))
                    n = len(mms)
                    for i, (lh, rh) in enumerate(mms):
                        nc.tensor.matmul(ps[0:np_parts, :], lh, rh, start=(i == 0),
                                         stop=(i == n - 1), skip_group_check=True)
                    if pmask is None:
                        nc.vector.tensor_add(dst[0:np_parts, d:d + 512],
                                             ps[0:np_parts, :],
                                             Sc[0:np_parts, o:o + 512])
                    else:
                        nc.vector.scalar_tensor_tensor(
                            out=dst[0:np_parts, d:d + 512], in0=ps[0:np_parts, :],
                            scalar=pmask, in1=Sc[0:np_parts, o:o + 512],
                            op0=OP.mult, op1=OP.add)

            def new_S(i):
                s = spl.tile([128, TWS], f16, name=f"S_{i % 3}", tag="S")
                nc.vector.memset(s[:, 0:SMS], 0.0)
                nc.vector.memset(s[:, SMS + L:TWS], 0.0)
                return s

            def new_F1(i):
                f = fpl_p.tile([128, TW], f16, name=f"F1_{i % 3}", tag="F1")
                nc.gpsimd.memset(f[:, 0:SM], 0.0)
                nc.gpsimd.memset(f[:, SM + L:TW], 0.0)
                return f

            # ---- extra F1 tiles at the image's first/last rows ----
            xs0 = new_S(0)
            score_tile(xs0, xfpl_sb, 0, 128, "xs0")
            xs1 = new_S(1)
            score_tile(xs1, xfpl_sb, 128, 128, "xs1")
            xf1a = big.tile([64, TW], f16)
            nc.vector.memset(xf1a[:, 0:SM], 0.0)
            nc.vector.memset(xf1a[:, SM + L:TW], 0.0)
            fuse1_tile(xf1a, xs0, None, None, None, 64, "xa")
            xf1b = big.tile([128, TW], f16)
            nc.vector.memset(xf1b[:, 0:SM], 0.0)
            nc.vector.memset(xf1b[:, SM + L:TW], 0.0)
            fuse1_tile(xf1b, xs1, None, None, None, 128, "xb")

            # ---- main pipeline over wide tiles ----
            Ss, F1s = {}, {}
            for t in range(11):
                if t <= 9:
                    Ss[t] = new_S(t + 2)
                    score_tile(Ss[t], fpl_sb, 128 * t, 128, f"s{t}")
                if 1 <= t:
                    u = t - 1
                    F1s[u] = new_F1(u)
                    fuse1_tile(F1s[u], Ss[u], Ss.get(u - 1), Ss.get(u + 1),
                               fmkp_sb[:, u:u + 1], 128, f"f{u}")
                if t < 3:
                    continue
                u = t - 2          # target wide tile 1..8
                jt = u - 1         # output tile 0..7
                strip = stp.tile([128, L], f16, name=f"strip_{jt % 2}", tag="strip")
                mct = stt_p.tile([128, 8], f32, name=f"mct_{jt}", tag="mct", bufs=2)
                for c8 in range(8):
                    o = SM + 512 * c8
                    ps = ps_sc.tile([128, 512], f32, name=f"pz_{jt}_{c8}", tag="ps")
                    mms = [
                        (slice(0, 512), 128, sh_sb[:, IDEN_i, :],
                         F1s[u][:, o:o + 512]),
                        (slice(0, 512), 128, sh_sb[:, SHUP_i, :],
                         F1s[u][:, o + 64:o + 576]),
                        (slice(0, 512), 128, sh_sb[:, SHDN_i, :],
                         F1s[u + 1][:, o + 64:o + 576]),
                        (slice(0, 512), 128, sh_sb[:, SHDN_i, :],
                         F1s[u][:, o - 64:o + 448]),
                        (slice(0, 512), 128, sh_sb[:, SHUP_i, :],
                         F1s[u - 1][:, o - 64:o + 448]),
                    ]
                    if u == 8:
                        mms.append((slice(0, 512), 64, sh_sb[0:64, CM2P_i, :],
                                    xf1a[:, 128 + 512 * c8:640 + 512 * c8]))
                    if u == 1:
                        mms.append((slice(0, 512), 128, sh_sb[:, CM2M_i, :],
                                    xf1b[:, 512 * c8:512 * c8 + 512]))
                    if c8 == 7:
                        mms.append((slice(448, 511), 128, sh_sb[:, SHUP_i, :],
                                    F1s[u][:, 65:128]))
                        mms.append((slice(448, 511), 128, sh_sb[:, SHDN_i, :],
                                    F1s[u + 1][:, 65:128]))
                        if u == 8:
                            mms.append((slice(448, 511), 64, sh_sb[0:64, CM2P_i, :],
                                        xf1a[:, 65:128]))
                    if c8 == 0:
                        mms.append((slice(1, 64), 128, sh_sb[:, SHDN_i, :],
                                    F1s[u][:, SM + L - 64:SM + L - 1]))
                        mms.append((slice(1, 64), 128, sh_sb[:, SHUP_i, :],
                                    F1s[u - 1][:, SM + L - 64:SM + L - 1]))
                        if u == 1:
                            mms.append((slice(1, 64), 128, sh_sb[:, CM2M_i, :],
                                        xf1b[:, SM + L - 64:SM + L - 1]))
                    n = len(mms)
                    for i, (psl, kp, lh, rh) in enumerate(mms):
                        nc.tensor.matmul(ps[:, psl], lh, rh, start=(i == 0),
                                         stop=(i == n - 1), skip_group_check=True)
                    # tt = (ps * 10) * mi in f32, then store strip = tt - max_c(tt)
                    # so the f16 rounding lands near 0 for the dominant weights
                    tt = stp.tile([128, 512], f32, name=f"tt_{jt}_{c8}",
                                  tag="tt", bufs=1)
                    nc.vector.scalar_tensor_tensor(
                        out=tt, in0=ps, scalar=SS,
                        in1=mir_sb[:, 512 * c8:512 * c8 + 512],
                        op0=OP.mult, op1=OP.mult)
                    nc.vector.tensor_reduce(out=mct[:, c8:c8 + 1], in_=tt,
                                            axis=AX.X, op=OP.max)
                    nc.vector.tensor_scalar(
                        out=strip[:, 512 * c8:512 * c8 + 512], in0=tt,
                        scalar1=mct[:, c8:c8 + 1], scalar2=None, op0=OP.subtract)

                # ---- softmax over l (chunked, exact f32 biases) ----
                M_t = stt_p.tile([128, 1], f32, name=f"M_{jt}", tag="M")
                nc.vector.tensor_reduce(out=M_t, in_=mct, axis=AX.X, op=OP.max)
                bct = stt_p.tile([128, 8], f32, name=f"bct_{jt}", tag="bct", bufs=2)
                nc.vector.tensor_scalar(out=bct, in0=mct, scalar1=M_t[:, :],
                                        scalar2=None, op0=OP.subtract)
                zct = stt_p.tile([128, 8], f32, name=f"zct_{jt}", tag="zct", bufs=2)
                for c8 in range(8):
                    nc.scalar.activation(out=strip[:, 512 * c8:512 * c8 + 512],
                                         in_=strip[:, 512 * c8:512 * c8 + 512],
                                         func=AF.Exp, bias=bct[:, c8:c8 + 1],
                                         scale=1.0, accum_out=zct[:, c8:c8 + 1])
                z_t = stt_p.tile([128, 1], f32, name=f"z_{jt}", tag="z")
                nc.vector.tensor_reduce(out=z_t, in_=zct, axis=AX.X, op=OP.add)
                zr_t = stt_p.tile([128, 1], f32, name=f"zr_{jt}", tag="zr")
                nc.vector.reciprocal(zr_t, z_t)
                # mask now; the 1/Z scale is applied at the output copy, so the
                # transposes need not wait for the denominator
                nc.vector.tensor_mul(strip, strip, mir_sb)

                # ---- epilogue: out_cols = P . bi ----
                accA = ps_ep.tile([128, 512], f32, name=f"accA_{jt}", tag="accA")
                accB = ps_ep.tile([128, 64], f32, name=f"accB_{jt}", tag="accB")
                for lt in range(32):
                    tp = ps_tp.tile([128, 128], f16, name=f"tp_{jt}_{lt}", tag="tp")
                    nc.tensor.transpose(tp, strip[:, lt * 128:(lt + 1) * 128],
                                        sh_sb[:, IDEN_i, :])
                    pcc = pcp.tile([128, 128], f16, name=f"pc_{jt}_{lt}", tag="pc",
                                   bufs=4)
                    nc.vector.tensor_copy(pcc, tp)
                    nc.tensor.matmul(accA, pcc, bir_sb[:, lt, 0:512],
                                     start=(lt == 0), stop=(lt == 31),
                                     skip_group_check=True)
                    nc.tensor.matmul(accB, pcc, bir_sb[:, lt, 512:576],
                                     start=(lt == 0), stop=(lt == 31),
                                     skip_group_check=True)
                ot = osb.tile([128, K], f16, name=f"ot_{jt % 2}", tag="ot", bufs=1)
                nc.vector.tensor_scalar_mul(ot[:, 0:512], accA, zr_t[:, :])
                nc.vector.tensor_scalar_mul(ot[:, 512:576], accB, zr_t[:, :])
                nc.sync.dma_start(out=out_d[jt], in_=ot)
    nc.finalize()
    return nc


# ---------------- cached jitted runner ----------------

NITER = 1025  # loop count of the timing NEFF (amortizes dispatch + ship)


def _make_runner(nc):
    import jax
    from concourse import bass2jax as b2j
    b2j.install_neuronx_cc_hook()

    partition_name = nc.partition_id_tensor.name if nc.partition_id_tensor else None
    in_names, out_names, out_avals, zero_outs = [], [], [], []
    for alloc in nc.m.functions[0].allocations:
        if not isinstance(alloc, mybir.MemoryLocationSet):
            continue
        name = alloc.memorylocations[0].name
        if alloc.kind == "ExternalInput":
            if name != partition_name:
                in_names.append(name)
        elif alloc.kind == "ExternalOutput":
            shape = tuple(alloc.tensor_shape)
            dtype = mybir.dt.np(alloc.dtype)
            out_names.append(name)
            out_avals.append(jax.core.ShapedArray(shape, dtype))
            zero_outs.append(np.zeros(shape, dtype))
    n_params = len(in_names)
    n_outs = len(out_avals)
    all_names = in_names + out_names + ([partition_name] if partition_name else [])
    donate = tuple(range(n_params, n_params + n_outs))

    def _body(*args):
        operands = list(args)
        if partition_name is not None:
            operands.append(b2j.partition_id_tensor())
        outs = b2j._bass_exec_p.bind(
            *operands, out_avals=tuple(out_avals), in_names=tuple(all_names),
            out_names=tuple(out_names), lowering_input_output_aliases=(),
            sim_require_finite=True, sim_require_nnan=True, nc=nc)
        return tuple(outs)

    devices = jax.devices()[:8]
    mesh = b2j.Mesh(np.asarray(devices), ("core",))
    in_specs = (b2j.PartitionSpec("core"),) * (n_params + n_outs)
    out_specs = (b2j.PartitionSpec("core"),) * n_outs
    sharded = jax.jit(
        b2j.shard_map(_body, mesh=mesh, in_specs=in_specs, out_specs=out_specs,
                      check_rep=False),
        donate_argnums=donate, keep_unused=True)
    return dict(fn=sharded,
                in_names=in_names, out_names=out_names, out_avals=out_avals,
                zero_outs=zero_outs, n_params=n_params, n_outs=n_outs)


def _run_device(nc, in_maps):
    import jax
    if "runner" not in _cached:
        _cached["runner"] = _make_runner(nc)
    R = _cached["runner"]

    gin = [np.concatenate([np.asarray(in_maps[c][name])[None] for c in range(8)], axis=0)
           .reshape(8 * np.asarray(in_maps[0][name]).shape[0],
                    *np.asarray(in_maps[0][name]).shape[1:])
           for name in R["in_names"]]

    def zeros():
        return [np.zeros((8 * z.shape[0], *z.shape[1:]), z.dtype)
                for z in R["zero_outs"]]

    # production call: ships inputs, runs once, fetch results
    t0p = time.perf_counter()
    ret = R["fn"](*gin, *zeros())
    jax.block_until_ready(ret)
    t_prod = time.perf_counter() - t0p
    results = []
    for c in range(8):
        rd = {}
        for i, name in enumerate(R["out_names"]):
            av = R["out_avals"][i]
            rd[name] = np.asarray(ret[i]).reshape(8, *av.shape)[c]
        results.append(rd)

    # timing: full dispatches (ship + exec) of the 1-iteration NEFF vs an
    # NITER-loop NEFF; ship/RPC cancel in the delta.  With BASS_SELF_TIME
    # (set by test.py) extra reps tighten the minimum; without it the
    # production dispatch doubles as t1 so the graded direct-call path adds
    # only the loop-NEFF dispatches.
    exec_ns = None
    reps = 3 if os.environ.get("BASS_SELF_TIME") else 2
    try:
        def timed(fn, n):
            best = None
            for _ in range(n):
                t0 = time.perf_counter()
                r = fn(*gin, *zeros())
                jax.block_until_ready(r)
                dt = time.perf_counter() - t0
                best = dt if best is None else min(best, dt)
            return best, r
        t1, _ = timed(R["fn"], reps)
        t1 = min(t1, t_prod)
        exec_ns = int(t1 * 1e9)  # last resort: one full dispatch
        for attempt in range(3):
            try:
                if "runnerN" not in _cached:
                    _cached["runnerN"] = _make_runner(_build_nc(NITER))
                tN, r = timed(_cached["runnerN"]["fn"], reps)
                if not np.isfinite(np.asarray(r[0]).astype(np.float32)).all():
                    raise RuntimeError("timing NEFF produced non-finite values")
                d = int((tN - t1) / (NITER - 1) * 1e9)
                exec_ns = d if d > 0 else int(tN / NITER * 1e9)
                break
            except Exception as e:
                _cached["timing_error"] = repr(e)
                _cached.pop("runnerN", None)
    except Exception as e:
        _cached["timing_error"] = repr(e)
    return results, exec_ns


# ---------------- numpy fallback (exact reference emulation) ----------------

def _img_patches_kp(img, edge_pad):
    """(c,h,w) image -> (L, K) patch matrix in k' = s*64+c order."""
    mode = 'edge' if edge_pad else 'constant'
    xp = np.pad(img, ((0, 0), (PAD, PAD), (PAD, PAD)), mode=mode)
    p = np.stack([xp[:, dy:dy + H, dx:dx + W] for dy in range(PS) for dx in range(PS)],
                 axis=0)                     # (9, c, h, w)
    return p.reshape(PS * PS * C, L).T.copy()  # k' = s*64+c


def _host_numpy(f_o, b_o, mask_o):
    B = f_o.shape[0]
    outs = []
    for e in range(B):
        bi = _img_patches_kp(b_o[e], True)
        fpm = _img_patches_kp(f_o[e], False)
        bnorm = np.maximum(np.sqrt((bi * bi).sum(1)), 1e-4)
        bsi = bi / bnorm[:, None]
        score = bsi @ fpm.T                      # (L_bg, L_fg)

        def diag_fuse(S):
            F = S.copy()
            F[1:, 1:] += S[:-1, :-1]
            F[:-1, :-1] += S[1:, 1:]
            return F
        S = diag_fuse(score)
        S = S.reshape(H, W, H, W).transpose(1, 0, 3, 2).reshape(L, L)
        S = diag_fuse(S)
        S = S.reshape(W, H, W, H).transpose(1, 0, 3, 2).reshape(L, L)
        mp = np.pad(mask_o[e][0], PAD)
        mmean = sum(mp[dy:dy + H, dx:dx + W] for dy in range(PS) for dx in range(PS)) / 9.0
        mi = (mmean == 0.0).astype(np.float32).reshape(L)
        S = S * mi[:, None] * np.float32(SS)
        S -= S.max(axis=0, keepdims=True)
        P = np.exp(S, dtype=np.float32)
        P /= P.sum(axis=0, keepdims=True)
        P *= mi[:, None]
        tmp = (bi.T @ P).reshape(PS * PS, C, H, W)
        acc = np.zeros((C, H + 2, W + 2), np.float32)
        for dy in range(PS):
            for dx in range(PS):
                acc[:, dy:dy + H, dx:dx + W] += tmp[dy * PS + dx]
        outs.append(acc[:, 1:1 + H, 1:1 + W] / np.float32(4.0))
    return np.stack(outs).astype(np.float32)


# ---------------- entry point ----------------

def kernel(f_o, b_o, mask_o):
    f_o = np.asarray(f_o, dtype=np.float32)
    b_o = np.asarray(b_o, dtype=np.float32)
    mask_o = np.asarray(mask_o, dtype=np.float32)
    B = f_o.shape[0]
    if "nc" not in _cached:
        _cached["nc"] = _build_nc()
    nc = _cached["nc"]

    in_maps = []
    for core in range(8):
        e, blk = divmod(core, NBLK)
        j0 = blk * JB
        r0 = 16 * blk
        b16 = b_o[e].astype(np.float16)
        f16 = f_o[e].astype(np.float16)
        FZ = np.zeros((C, H + 2, W + 2), np.float16)
        FZ[:, 1:65, 1:65] = f16
        slab = np.zeros((C, 24, 66), np.float16)
        lo = r0 - 3
        s_lo, s_hi = max(lo, 0), min(lo + 24, 66)
        slab[:, s_lo - lo:s_hi - lo, :] = FZ[:, s_lo:s_hi, :]
        fzx = np.zeros((C, 8, 66), np.float16)
        fzx[:, 0:4] = FZ[:, 0:4]
        fzx[:, 4:8] = FZ[:, 62:66]
        bpad = np.pad(b16, ((0, 0), (1, 1), (1, 1)), mode='edge')
        mp = np.pad(mask_o[e][0], PAD)
        mmean = sum(mp[dy:dy + H, dx:dx + W]
                    for dy in range(PS) for dx in range(PS)) / 9.0
        mi = (mmean == 0.0).astype(np.float16).reshape(1, L)
        fmsk = np.zeros((1, FPLW), np.float16)
        qs = np.arange(FPLW) + j0 - PADF
        fmsk[0, (qs >= 0) & (qs < L)] = 1.0
        fmskp = fmsk[0].reshape(NT, 128).T.astype(np.float32)
        sh = _SH_BASE.copy()
        if blk != NBLK - 1:
            sh[:, CM2P_i, :] = 0.0
        if blk != 0:
            sh[:, CM2M_i, :] = 0.0
        in_maps.append({
            "BPAD": bpad,
            "FZS": slab,
            "FZX": fzx,
            "FMSK": fmsk,
            "FMSKP": fmskp,
            "MI1": mi,
            "ONES": np.ones((128, 128), np.float16),
            "SHIFTS": sh,
        })

    _cached["last_in_maps"] = in_maps
    try:
        results, exec_ns = _run_device(nc, in_maps)
        _cached["exec_time_ns"] = exec_ns
    except Exception:
        try:
            res = run_bass_kernel_spmd(nc, in_maps, list(range(8)))
            results = res.results
            _cached["exec_time_ns"] = res.exec_time_ns
        except Exception:
            # last resort: numpy emulation of the device pipeline
            _cached["exec_time_ns"] = None
            return _host_numpy(f_o, b_o, mask_o)

    outs = []
    for e in range(B):
        acc = np.zeros((C, H + 2, W + 2), np.float32)
        for blk in range(NBLK):
            tmpT = results[e * NBLK + blk]["TMPT"].reshape(JB, K)
            t9 = tmpT.astype(np.float32).reshape(JB, PS * PS, C)
            y0 = blk * 16
            for dy in range(PS):
                for dx in range(PS):
                    sidx = dy * PS + dx
                    acc[:, y0 + dy: y0 + dy + 16, dx:dx + W] += \
                        t9[:, sidx, :].T.reshape(C, 16, W)
        outs.append(acc[:, 1:1 + H, 1:1 + W] / np.float32(4.0))
    return np.stack(outs).astype(np.float32)
